# revision 50
# baseline (speedup 1.0000x reference)
"""Multi-head attention (dense_transformer) on 8 TRN2 NeuronCores.

Sharding: data-parallel over batch (2) x tensor-parallel over head groups
(16 heads -> 4 groups of 4). Core (b, g) computes, for batch b:
  Q/K/V for its 4 heads (x @ w_qkv columns), per-head softmax(QK^T/sqrt(d))V,
  and the partial projection  attn_out_g @ w_proj[rows of g]  (+ b_proj/4).
The host sums the 4 partial projections per batch (the "all-reduce after
proj" of the sharding hint, done at gather time) and stacks the 2 batches.

Key engine balance (v3): softmax exp is the wall (~16.8M elems/core, 1
elem/lane/cycle on ACT). The exp tiles are split between the Scalar
engine (true exp via table) and the Vector engine (Schraudolph-style
exp: bf16 bit pattern = int16(A*s + B), computed as a single fused
tensor_scalar mult+add with int16 output, bitcast to bf16; ~3% max rel
err). This nearly doubles softmax throughput, removes the exp-wait gaps
that kept the PE's HAM clock-gate cold (K=4/8), and the softmax
denominators use nc.vector.reciprocal_approx_fast instead of ln/exp so
the ACT engine needs exactly one activation-table load (exp set, which
also contains Copy for the PSUM evacuations).
"""

import numpy as np

DIM = 1024
NUM_HEADS = 16
HEAD_DIM = 64
SCALE = HEAD_DIM ** -0.5
B = 2
N = 2048
NCORES = 8
HPG = 4               # heads per group (tensor-parallel degree 4)
GD = HPG * HEAD_DIM   # 256 dims per head group
CC = DIM // 128       # 8 contraction chunks over the model dim
TB = N // 128         # 16 token blocks
NQ = N // 512         # 4 query chunks
KB = N // 128         # 16 key blocks

_CACHE = {}


def _build_nc():
    from contextlib import ExitStack

    import concourse.tile as tile
    from concourse import bacc, mybir

    f32 = mybir.dt.float32
    bf16 = mybir.dt.bfloat16
    i16 = mybir.dt.int16
    EXP = mybir.ActivationFunctionType.Exp
    MULT = mybir.AluOpType.mult
    ADD = mybir.AluOpType.add

    # Schraudolph constants: bf16(2^(z)) bits ~= int16(128*(z + 127 - c)),
    # z = s * SCALE * log2(e). c centers the mantissa-linearization error
    # (+-3% rel).
    LOG2E = float(np.log2(np.e))
    SCH_A = 128.0 * LOG2E * SCALE
    SCH_B = 128.0 * (127.0 - 0.0430)

    nc = bacc.Bacc("TRN2", target_bir_lowering=False, debug=False,
                   enable_asserts=False)

    # Matmul operands arrive pre-rounded to bf16 and pre-rearranged to the
    # on-chip layout on the host, so every input DMA is contiguous.
    xt = nc.dram_tensor("xt", [DIM, N], bf16, kind="ExternalInput").ap()
    wq = nc.dram_tensor("wq", [128, CC, GD], bf16, kind="ExternalInput").ap()
    wk = nc.dram_tensor("wk", [128, CC, GD], bf16, kind="ExternalInput").ap()
    wv = nc.dram_tensor("wv", [128, CC, GD], bf16, kind="ExternalInput").ap()
    wp = nc.dram_tensor("wp", [128, 2, DIM], bf16, kind="ExternalInput").ap()
    # Output partials leave in bf16 (the host all-reduce sums them in fp32);
    # this halves the output DMA, which dominates the kernel tail.
    out = nc.dram_tensor("out", [N, DIM], bf16, kind="ExternalOutput").ap()

    with tile.TileContext(nc) as tc, ExitStack() as ctx:
        big = ctx.enter_context(tc.tile_pool(name="big", bufs=1))
        xts = ctx.enter_context(tc.tile_pool(name="xts", bufs=CC))
        pts = ctx.enter_context(tc.tile_pool(name="pts", bufs=6))
        outst = ctx.enter_context(tc.tile_pool(name="outst", bufs=2))
        small = ctx.enter_context(tc.tile_pool(name="small", bufs=2))
        # PSUM: a 3-deep rotation of [128,1024] S^T tiles (6 banks; also
        # carries the QKV and proj matmul outputs) + 2 PV accumulator banks
        # for the pass's head pair. Heads run as two sequential passes of 2.
        stps = ctx.enter_context(tc.tile_pool(name="stps", bufs=3, space="PSUM"))
        psum = ctx.enter_context(tc.tile_pool(name="psum", bufs=1, space="PSUM"))

        # ---- resident inputs ------------------------------------------------
        # DMA order matters for the pipeline ramp: wk + the first xt chunks
        # feed the first K-tile matmuls ~5us in; wp is needed last.
        wk_sb = big.tile([128, CC, GD], bf16, tag="wk")
        nc.sync.dma_start(wk_sb[:], wk)
        xt_t = [xts.tile([128, N], bf16, tag="xt", name=f"xt{cc}")
                for cc in range(CC)]
        for cc in range(CC):
            nc.sync.dma_start(xt_t[cc][:, 0:512], xt[cc * 128:(cc + 1) * 128, 0:512])
        wq_sb = big.tile([128, CC, GD], bf16, tag="wq")
        nc.sync.dma_start(wq_sb[:], wq)
        wv_sb = big.tile([128, CC, GD], bf16, tag="wv")
        nc.sync.dma_start(wv_sb[:], wv)
        for nq in range(1, NQ):
            for cc in range(CC):
                nc.sync.dma_start(
                    xt_t[cc][:, nq * 512:(nq + 1) * 512],
                    xt[cc * 128:(cc + 1) * 128, nq * 512:(nq + 1) * 512])
        wp_sb = big.tile([128, 2, DIM], bf16, tag="wp")
        nc.sync.dma_start(wp_sb[:], wp)
        qt_sb = big.tile([128, 2, N], bf16, tag="qt")   # Q^T: [d, tok]
        kt_sb = big.tile([128, 2, N], bf16, tag="kt")   # K^T: [d, tok]
        # V stored per (token-block, head) as [V_h | ones] (128 cols): the PV
        # matmul uses the whole 128-col block as lhsT (M=128) so PSUM rows
        # 0-63 get O^T_h and rows 64-127 get the softmax denom replicated 64x.
        v_sb = big.tile([128, TB, HPG, 128], bf16, tag="v")
        nc.vector.memset(v_sb[:, :, :, 64:128], 1.0)
        ot_sb = big.tile([128, 2, N], bf16, tag="ot")   # attn-out^T: [d, tok]

        # psum->sbuf evacuations alternate between ScalarE and VectorE.
        qkv_cp = [0]

        def evac_copy(dst, src):
            qkv_cp[0] ^= 1
            (nc.scalar.copy if qkv_cp[0] else nc.vector.tensor_copy)(dst, src)

        # ---- phase 1: QKV ---------------------------------------------------
        # Only the tiles the first attention pass needs are emitted up front
        # (K/Q for head pair 0, the first 6 V token-blocks); the rest go into
        # a work queue drained at the kb-group boundaries of the first passes,
        # so the QKV matmuls fill PE stalls while the exp pipeline ramps.
        qkv_i = [0]

        def emit_qk_tile(w_sb, dst, mb, nq):
            ps = stps.tile([128, 512], f32, tag="stp",
                           name=f"qkps{qkv_i[0]}")
            qkv_i[0] += 1
            for cc in range(CC):
                nc.tensor.matmul(
                    ps[:],
                    w_sb[:, cc, mb * 128:(mb + 1) * 128],
                    xt_t[cc][:, nq * 512:(nq + 1) * 512],
                    start=(cc == 0), stop=(cc == CC - 1),
                )
            evac_copy(dst[:, mb, nq * 512:(nq + 1) * 512], ps[:])

        def emit_v_tile(tb):
            ps = stps.tile([128, 512], f32, tag="stp", name=f"vps{tb}")
            for cc in range(CC):
                nc.tensor.matmul(
                    ps[:, 0:GD],
                    xt_t[cc][:, tb * 128:(tb + 1) * 128],
                    wv_sb[:, cc, :],
                    start=(cc == 0), stop=(cc == CC - 1),
                )
            evac_copy(
                v_sb[:, tb, :, 0:64],
                ps[:, 0:GD].rearrange("p (h d) -> p h d", h=HPG))

        # nq-major so the prologue consumes xt chunks in DMA arrival order
        for nq in range(NQ):
            emit_qk_tile(wk_sb, kt_sb, 0, nq)
            emit_qk_tile(wq_sb, qt_sb, 0, nq)
            for tb in range(4 * nq, min(4 * nq + 4, 11)):
                emit_v_tile(tb)
        qkv_work = [(1, emit_v_tile, (tb,)) for tb in range(11, TB)]
        for nq in range(NQ):
            qkv_work.append((2, emit_qk_tile, (wk_sb, kt_sb, 1, nq)))
        for nq in range(NQ):
            qkv_work.append((2, emit_qk_tile, (wq_sb, qt_sb, 1, nq)))

        # ---- phase 2+3: attention with fused projection ---------------------
        # Two passes of one head pair each (PSUM budget: 2 acc banks + 3-deep
        # S^T rotation). Per k-block one [128,1024] S^T tile (the pair's two
        # matmuls run row-tiled concurrently on the PE array halves); its exp
        # goes to ACT or DVE per the Bresenham split. The projection of
        # q-chunk nq (emittable only in the second pass, when both ot halves
        # exist) is deferred into the kb-loop of q-chunk nq+1 (kb 5..8) so the
        # PE has dense queued work while the normalize chain drains.
        pending = []   # deferred projection chunks: list of tb
        prefilled = {}
        proj_cp = [0]

        def emit_proj_chunk(tb):
            ps = stps.tile([128, 2, 512], f32, tag="stp", name=f"pjps{tb}")
            for nb in range(2):
                for dc in range(2):
                    nc.tensor.matmul(
                        ps[:, nb, :],
                        ot_sb[:, dc, tb * 128:(tb + 1) * 128],
                        wp_sb[:, dc, nb * 512:(nb + 1) * 512],
                        start=(dc == 0), stop=(dc == 1),
                    )
            ob = outst.tile([128, 1024], bf16, tag="ob", name=f"ob{tb}")
            proj_cp[0] ^= 1
            cp = nc.scalar.copy if proj_cp[0] else nc.vector.tensor_copy
            cp(ob[:], ps[:].rearrange("p a b -> p (a b)"))
            nc.sync.dma_start(out[tb * 128:(tb + 1) * 128, :], ob[:])

        # k-blocks run in groups of 3 so the PE stays in one tiling mode for
        # 3 S^T pairs, then one mode switch covers 6 PV matmuls (mode switches
        # drain the PE array; per-kb alternation costs ~300ns/kb).
        GRP = 3

        def finish_pass(carry, prefill_tbs=()):
            # trailing PVs of the pass's last kb-group, optional dc0 prefill
            # of the final projection chunks, then the normalize epilogue:
            # ACT Copy shifts the packed denominator rows 64-127 down to
            # partitions 0-63 (cross-partition read shifts are only proven
            # for ACT; custom DVE ops mis-handle them), the DVE
            # approx-reciprocal runs same-partition, and O^T_h * (1/l) lands
            # in ot_sb (DVE write-shift, proven).
            c_pr, c_nq, c_accs, c_pts = carry
            c_qsl = slice(c_nq * 512, (c_nq + 1) * 512)
            for kb, pt in c_pts:
                for hh in range(2):
                    nc.tensor.matmul(
                        c_accs[hh][:], v_sb[:, kb, 2 * c_pr + hh, :],
                        pt[:, hh * 512:(hh + 1) * 512],
                        start=(kb == 0), stop=(kb == KB - 1),
                    )
            for tb in prefill_tbs:
                ps = stps.tile([128, 2, 512], f32, tag="stp", name=f"pjf{tb}")
                for nb in range(2):
                    nc.tensor.matmul(
                        ps[:, nb, :],
                        ot_sb[:, 0, tb * 128:(tb + 1) * 128],
                        wp_sb[:, 0, nb * 512:(nb + 1) * 512],
                        start=True, stop=False,
                    )
                prefilled[tb] = ps
            for hh in range(2):
                po = 64 * hh
                dl = small.tile([64, 512], f32, tag="dl",
                                name=f"dl{c_pr}_{hh}_{c_nq}")
                nc.scalar.copy(dl[:], c_accs[hh][64:128, :])
                rec = small.tile([64, 512], f32, tag="rec",
                                 name=f"rec{c_pr}_{hh}_{c_nq}")
                nc.vector.reciprocal_approx_fast(rec[:], dl[:])
                nc.vector.tensor_mul(
                    ot_sb[po:po + 64, c_pr, c_qsl], c_accs[hh][0:64, :], rec[:])

        for pr in range(2):
            for nq in range(NQ):
                qsl = slice(nq * 512, (nq + 1) * 512)
                accs = [psum.tile([128, 512], f32, tag=f"acc{hh}",
                                  name=f"acc{pr}_{hh}_{nq}")
                        for hh in range(2)]
                prev_pts = []   # PVs run one kb-group behind the S^T/exp
                for g0 in range(0, KB, GRP):
                    kbs = range(g0, min(g0 + GRP, KB))
                    grp_pts = []
                    for kb in kbs:
                        stp = stps.tile([128, 1024], f32, tag="stp",
                                        name=f"stp{pr}_{nq}_{kb}")
                        for hh in range(2):
                            po = 64 * hh
                            nc.tensor.matmul(
                                stp[:, hh * 512:(hh + 1) * 512],
                                kt_sb[po:po + 64, pr, kb * 128:(kb + 1) * 128],
                                qt_sb[po:po + 64, pr, qsl],
                                start=True, stop=True,
                            )
                        pt = pts.tile([128, 1024], bf16, tag="pt",
                                      name=f"pt{pr}_{nq}_{kb}")
                        # strict ACT/DVE alternation (9/7): same-engine bursts
                        # within a kb-group stall the S^T pipeline. Last
                        # k-block on ACT so the DVE is free for the epilogue.
                        if kb % 2 == 1 and kb < KB - 1:
                            nc.vector.tensor_scalar(
                                pt[:].bitcast(i16), stp[:], SCH_A, SCH_B,
                                MULT, ADD)
                        else:
                            nc.scalar.activation(pt[:], stp[:], EXP, scale=SCALE)
                        grp_pts.append((kb, pt))
                    if pending and 9 <= g0 <= 12:
                        emit_proj_chunk(pending.pop(0))
                        emit_proj_chunk(pending.pop(0))
                    for kb, pt in prev_pts:
                        for hh in range(2):
                            nc.tensor.matmul(
                                accs[hh][:], v_sb[:, kb, 2 * pr + hh, :],
                                pt[:, hh * 512:(hh + 1) * 512],
                                start=(kb == 0), stop=(kb == KB - 1),
                            )
                    prev_pts = grp_pts
                    # drain deferred QKV work after the PVs: the leftover V /
                    # K,Q head-pair-1 tiles fill the PE while exps ramp
                    if qkv_work and pr == 0:
                        budget = qkv_work[0][0]
                        for _ in range(budget):
                            if qkv_work:
                                _, fn, args = qkv_work.pop(0)
                                fn(*args)
                last = (pr == 1 and nq == NQ - 1)
                finish_pass(
                    (pr, nq, accs, prev_pts),
                    prefill_tbs=range(4 * nq, 4 * nq + 3) if last else ())
                if pr == 1:
                    assert not pending
                    pending = [4 * nq + i for i in range(4)]
        # final q-chunk's projection: finish the prefilled dc0 accumulations,
        # split copies across both engines, DMA per 512-col half
        for tb in pending:
            ps = prefilled.get(tb)
            dcs = (1,) if ps is not None else (0, 1)
            if ps is None:
                ps = stps.tile([128, 2, 512], f32, tag="stp", name=f"pjps{tb}")
            for nb in range(2):
                for dc in dcs:
                    nc.tensor.matmul(
                        ps[:, nb, :],
                        ot_sb[:, dc, tb * 128:(tb + 1) * 128],
                        wp_sb[:, dc, nb * 512:(nb + 1) * 512],
                        start=(dc == 0), stop=(dc == 1),
                    )
            ob = outst.tile([128, 1024], bf16, tag="ob", name=f"obf{tb}")
            for nb in range(2):
                cp = nc.scalar.copy if nb == 0 else nc.vector.tensor_copy
                cp(ob[:, nb * 512:(nb + 1) * 512], ps[:, nb, :])
                nc.sync.dma_start(
                    out[tb * 128:(tb + 1) * 128, nb * 512:(nb + 1) * 512],
                    ob[:, nb * 512:(nb + 1) * 512])

    nc.compile()
    return nc


def get_nc():
    if "nc" not in _CACHE:
        _CACHE["nc"] = _build_nc()
    return _CACHE["nc"]


def to_bf16(a):
    import ml_dtypes
    return np.ascontiguousarray(np.asarray(a, dtype=np.float32)).astype(ml_dtypes.bfloat16)


def make_in_maps(x, w_qkv, w_proj, b_proj):
    x = np.ascontiguousarray(np.asarray(x, dtype=np.float32))
    w_qkv = np.asarray(w_qkv, dtype=np.float32)
    w_proj = np.asarray(w_proj, dtype=np.float32)
    b_proj = np.asarray(b_proj, dtype=np.float32)

    wr = w_qkv.reshape(DIM, 3, NUM_HEADS, HEAD_DIM)
    xts = [to_bf16(x[b].T) for b in range(B)]

    def chipw(a):       # [DIM, GD] -> [128, CC, GD] (cc p) d -> p cc d
        return to_bf16(a.reshape(CC, 128, GD).transpose(1, 0, 2))

    in_maps = []
    for core in range(NCORES):
        b, g = divmod(core, HPG)
        h0, h1 = HPG * g, HPG * (g + 1)
        in_maps.append({
            "xt": xts[b],
            "wq": chipw(wr[:, 0, h0:h1, :].reshape(DIM, GD)),
            "wk": chipw(wr[:, 1, h0:h1, :].reshape(DIM, GD)),
            "wv": chipw(wr[:, 2, h0:h1, :].reshape(DIM, GD)),
            "wp": to_bf16(w_proj[g * GD:(g + 1) * GD, :]
                          .reshape(2, 128, DIM).transpose(1, 0, 2)),
        })
    return in_maps


def gather_out(results, b_proj):
    parts = [np.asarray(r["out"], dtype=np.float32) for r in results]
    b_proj = np.asarray(b_proj, dtype=np.float32)
    return np.stack(
        [sum(parts[b * HPG:(b + 1) * HPG][1:], parts[b * HPG]) + b_proj
         for b in range(B)],
        axis=0,
    ).astype(np.float32)


def kernel(x, w_qkv, w_proj, b_proj):
    from concourse import bass_utils

    nc = get_nc()
    in_maps = make_in_maps(x, w_qkv, w_proj, b_proj)
    res = bass_utils.run_bass_kernel_spmd(nc, in_maps, core_ids=list(range(NCORES)))
    return gather_out(res.results, b_proj)
